# revision 21
# baseline (speedup 1.0000x reference)
"""Trainium2 Bass kernel for nn_AxonalConnections (gnn_message_passing).

Computes out[b,t] = sum_s adjacency[t,s] * mod[b,s],  mod = (1.5*E - 0.5) * spikes,
i.e. a batched mat-vec against a [16384, 16384] adjacency, reshaped to [32,128,128].

Sharding: adjacency row-shard (target dim) across 8 cores; spikes/E replicated;
each core produces out[:, t_shard] — pure output sharding, no collectives.

Two device paths:

* dense: bf16 GEMM, K=16384 accumulated in fp32 PSUM. Adjacency is host-side
  transposed/cast once so each core streams its [S, T/8] bf16 slab with
  fully-contiguous DMAs (fallback for arbitrary adjacency).

* sparse: when the adjacency's nonzeros all lie on the 9 conv-pattern
  diagonals (the generator's 3x3 message-passing graph), the GEMM is exactly a
  9-tap locally-connected stencil: out[b,t] = sum_k w9[t,k]*mod[b,t+d_k].
  The E-modulation is folded into w9 on the host (exact: the factor is a
  power-of-two scale in {1.0, -0.5}), and each core evaluates the stencil on
  a [4 t-quarters x 32 batch, 512] packed layout where every tap is a
  free-dim AP offset.

  All device data is fp16:
  - The DVE runs the 9 mult + 8 add chain in fp16 so tensor_tensor hits the
    2x_1P perf mode (~420 ns per [128,512] op vs ~690 ns fp32, measured).
    Tap offsets 129+d have mixed parity, so the padded spike slab ships
    TWICE at element offsets differing by one; each tap reads whichever copy
    makes its start offset 4-byte aligned.
  - Only 3 taps ship batch-replicated (to un-gate the DVE chain early). The
    other 6 ship compact ([4 quarters, 512] per tap + a 0/1 selector block)
    and are broadcast across the 32 batch lanes on-chip: a [4,128] selector
    matmul on the otherwise-idle TensorE writes the replicated tap to PSUM
    (exact: one nonzero per column), and ScalarE copies PSUM->SBUF fp16
    (exact round trip). This keeps total per-core DMA at ~0.8 MB — measured
    on this part, every queued transfer's completion smears toward
    (first-packet + total_bytes/210GB/s), so minimizing queued bytes is what
    actually moves the DVE start time — and lets DMA, TensorE, ScalarE and
    the DVE pipeline per tap with no mid-chain starvation.

  Measured on hardware: 28.4 us (fp32 17-op chain baseline) -> 23.0 us.
  Of the 23 us, ~10.5 us is fixed NEFF-wrapper overhead inside the measured
  window (per-engine semaphore-file clears + barriers + const-pool init +
  HBM write receipt), ~5.5 us is input-DMA landing, ~6.3 us is the DVE chain.
"""

import sys

if "/opt/trn_rl_repo" not in sys.path:
    sys.path.insert(0, "/opt/trn_rl_repo")

from contextlib import ExitStack

import ml_dtypes
import numpy as np

B = 32
H = 128
W = 128
S = H * W            # 16384
NCORES = 8
TL = S // NCORES     # 2048 t-columns per core
KC = S // 128        # 128 contraction chunks (dense path)
P = 128

# sparse path geometry: 3x3 conv neighborhood offsets in flattened index space
DIAG_OFFSETS = [di * W + dj for di in (-1, 0, 1) for dj in (-1, 0, 1)]
NTAP = len(DIAG_OFFSETS)
NQ = 4               # t-quarters packed on partitions: 4*32 = 128
QT = TL // NQ        # 512 t per quarter
PADE = 130           # spike padding on each side (covers |d|<=129 both parities)
QW = 772             # per-parity spike slab width: 258 halo span + 512 + slack
NDMA = 2             # taps shipped batch-replicated; the rest are PE-broadcast
NBC = NTAP - NDMA    # taps broadcast on-chip from compact form
WCW = NBC * QT + P   # wcsel row width: compact taps + selector block
# wrep carries the two taps consumed LAST (its DMA lands last on the sync
# ring); broadcast taps are listed in copy-production order, even-parity
# first (the even slab sB ships before the odd slab sA).
WREP_TAPS = [1, 4]
BC_TAPS = [7, 0, 2, 3, 5, 6, 8]
TAP_ORDER = BC_TAPS + WREP_TAPS

_progs = {}


def _build_dense():
    import concourse.tile as tile
    from concourse import bacc, mybir

    nc = bacc.Bacc("TRN2", target_bir_lowering=False, debug=False, num_devices=NCORES)
    f32 = mybir.dt.float32
    bf16 = mybir.dt.bfloat16

    adjt = nc.dram_tensor("adjt", [S, TL], bf16, kind="ExternalInput").ap()
    spt = nc.dram_tensor("spt", [P, KC, B], f32, kind="ExternalInput").ap()
    ef = nc.dram_tensor("ef", [P, KC], f32, kind="ExternalInput").ap()
    outt = nc.dram_tensor("out", [B, TL], f32, kind="ExternalOutput").ap()

    NT = TL // 512  # psum banks used for the output row block

    with tile.TileContext(nc) as tc:
        with ExitStack() as ctx:
            const = ctx.enter_context(tc.tile_pool(name="const", bufs=1))
            adj_pool = ctx.enter_context(tc.tile_pool(name="adj", bufs=10))
            psum = ctx.enter_context(tc.tile_pool(name="psum", bufs=1, space="PSUM"))
            outp = ctx.enter_context(tc.tile_pool(name="outp", bufs=1))

            sp_t = const.tile([P, KC, B], f32)
            nc.sync.dma_start(sp_t[:], spt[:])
            e_t = const.tile([P, KC], f32)
            nc.sync.dma_start(e_t[:], ef[:])
            fac = const.tile([P, KC], f32)
            # fac = 1.5*E - 0.5  (E in {0,1} -> {1.0, -0.5})
            nc.vector.tensor_scalar(
                fac[:], e_t[:], 1.5, -0.5,
                op0=mybir.AluOpType.mult, op1=mybir.AluOpType.add,
            )
            modt = const.tile([P, KC, B], bf16)
            for k in range(KC):
                nc.vector.tensor_scalar(
                    modt[:, k, :], sp_t[:, k, :], fac[:, k : k + 1], None,
                    op0=mybir.AluOpType.mult,
                )

            pts = [psum.tile([B, 512], f32, name=f"acc{j}") for j in range(NT)]
            for k in range(KC):
                at = adj_pool.tile([P, TL], bf16)
                nc.sync.dma_start(at[:], adjt[k * P : (k + 1) * P, :])
                for j in range(NT):
                    nc.tensor.matmul(
                        pts[j][:],
                        modt[:, k, :],
                        at[:, j * 512 : (j + 1) * 512],
                        start=(k == 0),
                        stop=(k == KC - 1),
                    )

            ot = outp.tile([B, TL], f32)
            for j in range(NT):
                nc.vector.tensor_copy(out=ot[:, j * 512 : (j + 1) * 512], in_=pts[j][:])
            nc.sync.dma_start(outt[:], ot[:])

    nc.compile()
    return nc


def _build_sparse():
    import concourse.tile as tile
    from concourse import bacc, mybir

    nc = bacc.Bacc("TRN2", target_bir_lowering=False, debug=False, num_devices=NCORES)
    f16 = mybir.dt.float16
    f32 = mybir.dt.float32

    # per-core inputs (host pre-packed, fp16):
    #   sB/sA: even-/odd-parity padded spike slabs (tap offsets 130+d / 129+d)
    #   wrep: [p, NDMA, QT] last-consumed taps, batch-replicated on host
    #   wcsel: [4, NBC*QT + 128] compact taps (row q) + 0/1 selector block
    sBd = nc.dram_tensor("sB", [P, QW], f16, kind="ExternalInput").ap()
    sAd = nc.dram_tensor("sA", [P, QW], f16, kind="ExternalInput").ap()
    wrep = nc.dram_tensor("wrep", [P, NDMA, QT], f16, kind="ExternalInput").ap()
    wcsel = nc.dram_tensor("wcsel", [4, WCW], f16, kind="ExternalInput").ap()
    # packed [32q+b, t] layout; host unpacks to [B, TL]
    outt = nc.dram_tensor("out", [P, QT], f16, kind="ExternalOutput").ap()

    with tile.TileContext(nc) as tc:
        with ExitStack() as ctx:
            pool = ctx.enter_context(tc.tile_pool(name="pool", bufs=1))
            psum = ctx.enter_context(tc.tile_pool(name="psum", bufs=1, space="PSUM"))

            # everything on the SP ring, in consumption order: the tiny
            # 4-partition wcsel first (its lone SDMA engine drains it before
            # its slab share, landing ~2 us earlier than on the ACT ring),
            # then even slab, odd slab, and the last-consumed replicated taps
            wcs = pool.tile([4, WCW], f16)
            nc.sync.dma_start(wcs[:], wcsel[:])
            sB = pool.tile([P, QW], f16)
            nc.sync.dma_start(sB[:], sBd[:])
            sA = pool.tile([P, QW], f16)
            nc.sync.dma_start(sA[:], sAd[:])
            wr = pool.tile([P, NDMA, QT], f16)
            nc.sync.dma_start(wr[:], wrep[:])

            sel = wcs[:, NBC * QT : NBC * QT + P]  # [4, 128] 0/1 selector
            # broadcast tap k across its quarter's 32 batch partitions:
            # psum[p, t] = sum_q sel[q, p] * wc_k[q, t] = wfold[(p//32)*QT+t, k]
            wts = {k: wr[:, j, :] for j, k in enumerate(WREP_TAPS)}
            for j, k in enumerate(BC_TAPS):
                ps = psum.tile([P, QT], f32, name=f"bc{j}")
                nc.tensor.matmul(
                    ps[:], sel, wcs[:, j * QT : (j + 1) * QT],
                    start=True, stop=True,
                )
                wk = pool.tile([P, QT], f16, name=f"w{j}")
                nc.scalar.copy(wk[:], ps[:])
                wts[k] = wk[:]

            acc = None
            for k in TAP_ORDER:
                d = DIAG_OFFSETS[k]
                # even-d taps read the B slab at offset 130+d; odd-d taps the
                # A slab at 129+d (both starts even -> 4-byte aligned)
                if d % 2 == 0:
                    sh = sB[:, 130 + d : 130 + d + QT]
                else:
                    sh = sA[:, 129 + d : 129 + d + QT]
                prod = pool.tile([P, QT], f16, name=f"prod{k}")
                nc.vector.tensor_tensor(prod[:], sh, wts[k], mybir.AluOpType.mult)
                if acc is None:
                    acc = prod
                else:
                    nxt = pool.tile([P, QT], f16, name=f"acc{k}")
                    nc.vector.tensor_tensor(
                        nxt[:], acc[:], prod[:], mybir.AluOpType.add
                    )
                    acc = nxt

            nc.sync.dma_start(outt[:], acc[:])

    nc.compile()
    return nc


def _get_prog(name):
    if name not in _progs:
        _progs[name] = {"dense": _build_dense, "sparse": _build_sparse}[name]()
    return _progs[name]


def _run(nc, in_maps, **kwargs):
    from concourse.bass_utils import run_bass_kernel_spmd

    return run_bass_kernel_spmd(nc, in_maps, core_ids=list(range(NCORES)), **kwargs)


def _extract_diagonals(adjacency):
    """W9[t, k] = adjacency[t, t + d_k] (0 where out of range).

    Returns (W9, exact) where exact means every nonzero of adjacency lies on
    those 9 diagonals, making the stencil reproduction of the GEMM exact.
    """
    t = np.arange(S)
    W9 = np.zeros((S, NTAP), np.float32)
    for k, d in enumerate(DIAG_OFFSETS):
        s = t + d
        valid = (s >= 0) & (s < S)
        W9[valid, k] = adjacency[t[valid], s[valid]]
    exact = np.count_nonzero(adjacency) == np.count_nonzero(W9)
    return W9, exact


def _prep_dense_inmaps(sp_flat, E_flat, adjacency):
    spt = np.ascontiguousarray(sp_flat.T.reshape(KC, P, B).transpose(1, 0, 2))
    ef = np.ascontiguousarray(E_flat.reshape(KC, P).T)
    adj_bf = adjacency.astype(ml_dtypes.bfloat16)
    in_maps = []
    for m in range(NCORES):
        adjt_m = np.ascontiguousarray(adj_bf[m * TL : (m + 1) * TL, :].T)
        in_maps.append({"adjt": adjt_m, "spt": spt, "ef": ef})
    return in_maps


def _prep_sparse_inmaps(sp_flat, E_flat, W9):
    # fold the E-modulation into the tap weights: exact because the factor is
    # the power-of-two scale {1.0, -0.5}
    fac = 1.5 * E_flat - 0.5
    t = np.arange(S)
    wfold = np.empty_like(W9)  # [S, 9]
    for k, d in enumerate(DIAG_OFFSETS):
        s = np.clip(t + d, 0, S - 1)
        wfold[:, k] = W9[:, k] * fac[s]
    wfold = wfold.astype(np.float16)

    sp_pad = np.zeros((B, S + 2 * PADE + 8), np.float16)
    sp_pad[:, PADE : PADE + S] = sp_flat

    # 0/1 selector shared across cores: sel[q, p] = (p // 32 == q)
    sel = (np.arange(P)[None, :] // B == np.arange(NQ)[:, None]).astype(np.float16)

    in_maps = []
    for m in range(NCORES):
        t0 = m * TL
        sBh = np.empty((NQ, B, QW), np.float16)
        sAh = np.empty((NQ, B, QW), np.float16)
        for q in range(NQ):
            tq = t0 + q * QT
            # A: sp_pad[:, tq+1+i] -> tap offset 129+d (odd d)
            # B: sp_pad[:, tq+i]   -> tap offset 130+d (even d)
            sAh[q] = sp_pad[:, tq + 1 : tq + 1 + QW]
            sBh[q] = sp_pad[:, tq : tq + QW]

        wslab = wfold[t0 : t0 + TL].reshape(NQ, QT, NTAP)
        # replicated taps: batch-replicated [4q*32b, NDMA, QT]
        wr = np.broadcast_to(
            wslab[:, None, :, WREP_TAPS].transpose(0, 1, 3, 2), (NQ, B, NDMA, QT)
        )
        # broadcast taps compact, in BC_TAPS (production) order
        wcsel = np.empty((NQ, WCW), np.float16)
        wcsel[:, : NBC * QT] = (
            wslab[:, :, BC_TAPS].transpose(0, 2, 1).reshape(NQ, NBC * QT)
        )
        wcsel[:, NBC * QT :] = sel
        in_maps.append(
            {
                "sB": sBh.reshape(P, QW),
                "sA": sAh.reshape(P, QW),
                "wrep": np.ascontiguousarray(wr).reshape(P, NDMA, QT),
                "wcsel": wcsel,
            }
        )
    return in_maps


def _gather_out(results):
    out = np.empty((B, S), np.float32)
    for m in range(NCORES):
        r = results[m]["out"]
        if r.shape == (P, QT):  # sparse path: unpack [32q+b, t] -> [b, q*QT+t]
            r = r.astype(np.float32).reshape(NQ, B, QT).transpose(1, 0, 2)
            r = r.reshape(B, TL)
        out[:, m * TL : (m + 1) * TL] = r
    return out


def kernel(spikes, E, adjacency):
    spikes = np.asarray(spikes, np.float32)
    E = np.asarray(E, np.float32)
    adjacency = np.asarray(adjacency, np.float32)
    sp_flat = spikes.reshape(B, S)
    E_flat = E.reshape(S)

    W9, exact = _extract_diagonals(adjacency)
    if exact:
        in_maps = _prep_sparse_inmaps(sp_flat, E_flat, W9)
        results = _run(_get_prog("sparse"), in_maps).results
    else:
        in_maps = _prep_dense_inmaps(sp_flat, E_flat, adjacency)
        results = _run(_get_prog("dense"), in_maps).results
    return _gather_out(results).reshape(B, H, W)


# revision 22
# speedup vs baseline: 1.2261x; 1.2261x over previous
"""Trainium2 Bass kernel for nn_AxonalConnections (gnn_message_passing).

Computes out[b,t] = sum_s adjacency[t,s] * mod[b,s],  mod = (1.5*E - 0.5) * spikes,
i.e. a batched mat-vec against a [16384, 16384] adjacency, reshaped to [32,128,128].

Sharding: adjacency row-shard (target dim) across 8 cores; spikes/E replicated;
each core produces out[:, t_shard] — pure output sharding, no collectives.

Two device paths:

* dense: bf16 GEMM, K=16384 accumulated in fp32 PSUM. Adjacency is host-side
  transposed/cast once so each core streams its [S, T/8] bf16 slab with
  fully-contiguous DMAs (fallback for arbitrary adjacency).

* sparse: when the adjacency's nonzeros all lie on the 9 conv-pattern
  diagonals (the generator's 3x3 message-passing graph), the GEMM is exactly a
  9-tap locally-connected stencil: out[b,t] = sum_k w9[t,k]*mod[b,t+d_k].
  The E-modulation is folded into w9 on the host (exact: the factor is a
  power-of-two scale in {1.0, -0.5}), and each core evaluates the stencil on
  a [4 t-quarters x 32 batch, 512] packed layout where every tap is a
  free-dim AP offset.

  All device data is fp16:
  - The DVE runs the 9 mult + 8 add chain in fp16 so tensor_tensor hits the
    2x_1P perf mode (~420 ns per [128,512] op vs ~690 ns fp32, measured).
    Tap offsets 129+d have mixed parity, so the padded spike slab ships
    TWICE at element offsets differing by one; each tap reads whichever copy
    makes its start offset 4-byte aligned.
  - Only 3 taps ship batch-replicated (to un-gate the DVE chain early). The
    other 6 ship compact ([4 quarters, 512] per tap + a 0/1 selector block)
    and are broadcast across the 32 batch lanes on-chip: a [4,128] selector
    matmul on the otherwise-idle TensorE writes the replicated tap to PSUM
    (exact: one nonzero per column), and ScalarE copies PSUM->SBUF fp16
    (exact round trip). This keeps total per-core DMA at ~0.8 MB — measured
    on this part, every queued transfer's completion smears toward
    (first-packet + total_bytes/210GB/s), so minimizing queued bytes is what
    actually moves the DVE start time — and lets DMA, TensorE, ScalarE and
    the DVE pipeline per tap with no mid-chain starvation.

  Measured on hardware: 28.4 us (fp32 17-op chain baseline) -> 23.0 us.
  Of the 23 us, ~10.5 us is fixed NEFF-wrapper overhead inside the measured
  window (per-engine semaphore-file clears + barriers + const-pool init +
  HBM write receipt), ~5.5 us is input-DMA landing, ~6.3 us is the DVE chain.
"""

import sys

if "/opt/trn_rl_repo" not in sys.path:
    sys.path.insert(0, "/opt/trn_rl_repo")

from contextlib import ExitStack

import ml_dtypes
import numpy as np

B = 32
H = 128
W = 128
S = H * W            # 16384
NCORES = 8
TL = S // NCORES     # 2048 t-columns per core
KC = S // 128        # 128 contraction chunks (dense path)
P = 128

# sparse path geometry: 3x3 conv neighborhood offsets in flattened index space
DIAG_OFFSETS = [di * W + dj for di in (-1, 0, 1) for dj in (-1, 0, 1)]
NTAP = len(DIAG_OFFSETS)
NQ = 4               # t-quarters packed on partitions: 4*32 = 128
QT = TL // NQ        # 512 t per quarter
PADE = 130           # spike padding on each side (covers |d|<=129 both parities)
QW = 772             # per-parity spike slab width: 258 halo span + 512 + slack
NDMA = 3             # taps shipped batch-replicated; the rest are PE-broadcast
NBC = NTAP - NDMA    # taps broadcast on-chip from compact form
WCW = NBC * QT + P   # wcsel row width: compact taps + selector block

_progs = {}


def _build_dense():
    import concourse.tile as tile
    from concourse import bacc, mybir

    nc = bacc.Bacc("TRN2", target_bir_lowering=False, debug=False, num_devices=NCORES)
    f32 = mybir.dt.float32
    bf16 = mybir.dt.bfloat16

    adjt = nc.dram_tensor("adjt", [S, TL], bf16, kind="ExternalInput").ap()
    spt = nc.dram_tensor("spt", [P, KC, B], f32, kind="ExternalInput").ap()
    ef = nc.dram_tensor("ef", [P, KC], f32, kind="ExternalInput").ap()
    outt = nc.dram_tensor("out", [B, TL], f32, kind="ExternalOutput").ap()

    NT = TL // 512  # psum banks used for the output row block

    with tile.TileContext(nc) as tc:
        with ExitStack() as ctx:
            const = ctx.enter_context(tc.tile_pool(name="const", bufs=1))
            adj_pool = ctx.enter_context(tc.tile_pool(name="adj", bufs=10))
            psum = ctx.enter_context(tc.tile_pool(name="psum", bufs=1, space="PSUM"))
            outp = ctx.enter_context(tc.tile_pool(name="outp", bufs=1))

            sp_t = const.tile([P, KC, B], f32)
            nc.sync.dma_start(sp_t[:], spt[:])
            e_t = const.tile([P, KC], f32)
            nc.sync.dma_start(e_t[:], ef[:])
            fac = const.tile([P, KC], f32)
            # fac = 1.5*E - 0.5  (E in {0,1} -> {1.0, -0.5})
            nc.vector.tensor_scalar(
                fac[:], e_t[:], 1.5, -0.5,
                op0=mybir.AluOpType.mult, op1=mybir.AluOpType.add,
            )
            modt = const.tile([P, KC, B], bf16)
            for k in range(KC):
                nc.vector.tensor_scalar(
                    modt[:, k, :], sp_t[:, k, :], fac[:, k : k + 1], None,
                    op0=mybir.AluOpType.mult,
                )

            pts = [psum.tile([B, 512], f32, name=f"acc{j}") for j in range(NT)]
            for k in range(KC):
                at = adj_pool.tile([P, TL], bf16)
                nc.sync.dma_start(at[:], adjt[k * P : (k + 1) * P, :])
                for j in range(NT):
                    nc.tensor.matmul(
                        pts[j][:],
                        modt[:, k, :],
                        at[:, j * 512 : (j + 1) * 512],
                        start=(k == 0),
                        stop=(k == KC - 1),
                    )

            ot = outp.tile([B, TL], f32)
            for j in range(NT):
                nc.vector.tensor_copy(out=ot[:, j * 512 : (j + 1) * 512], in_=pts[j][:])
            nc.sync.dma_start(outt[:], ot[:])

    nc.compile()
    return nc


def _build_sparse():
    import concourse.tile as tile
    from concourse import bacc, mybir

    nc = bacc.Bacc("TRN2", target_bir_lowering=False, debug=False, num_devices=NCORES)
    f16 = mybir.dt.float16
    f32 = mybir.dt.float32

    # per-core inputs (host pre-packed, fp16):
    #   spq2: [p=32q+b, 2*QW] dual-parity padded spike slabs
    #   wrep: [p, NDMA, QT] first NDMA taps, batch-replicated on host
    #   wcsel: [4, NBC*QT + 128] compact taps (row q) + 0/1 selector block
    spq2 = nc.dram_tensor("spq2", [P, 2 * QW], f16, kind="ExternalInput").ap()
    wrep = nc.dram_tensor("wrep", [P, NDMA, QT], f16, kind="ExternalInput").ap()
    wcsel = nc.dram_tensor("wcsel", [4, WCW], f16, kind="ExternalInput").ap()
    # packed [32q+b, t] layout; host unpacks to [B, TL]
    outt = nc.dram_tensor("out", [P, QT], f16, kind="ExternalOutput").ap()

    with tile.TileContext(nc) as tc:
        with ExitStack() as ctx:
            pool = ctx.enter_context(tc.tile_pool(name="pool", bufs=1))
            psum = ctx.enter_context(tc.tile_pool(name="psum", bufs=1, space="PSUM"))

            # wcsel on the ACT HWDGE queue (tiny, unblocks PE broadcasts);
            # spikes + replicated taps on the SP queue
            wcs = pool.tile([4, WCW], f16)
            nc.scalar.dma_start(wcs[:], wcsel[:])
            sp = pool.tile([P, 2 * QW], f16)
            nc.sync.dma_start(sp[:], spq2[:])
            wr = pool.tile([P, NDMA, QT], f16)
            nc.sync.dma_start(wr[:], wrep[:])

            sel = wcs[:, NBC * QT : NBC * QT + P]  # [4, 128] 0/1 selector
            # broadcast tap k across its quarter's 32 batch partitions:
            # psum[p, t] = sum_q sel[q, p] * wc_k[q, t] = wfold[(p//32)*QT+t, k]
            wts = [wr[:, k, :] for k in range(NDMA)]
            for j in range(NBC):
                ps = psum.tile([P, QT], f32, name=f"bc{j}")
                nc.tensor.matmul(
                    ps[:], sel, wcs[:, j * QT : (j + 1) * QT],
                    start=True, stop=True,
                )
                wk = pool.tile([P, QT], f16, name=f"w{j}")
                nc.scalar.copy(wk[:], ps[:])
                wts.append(wk[:])

            acc = None
            for k, d in enumerate(DIAG_OFFSETS):
                # tap offset in the A copy is 129+d (even for odd d); the B
                # copy sits QW elements later, one element earlier in t
                if (129 + d) % 2 == 0:
                    sh = sp[:, 129 + d : 129 + d + QT]
                else:
                    sh = sp[:, QW + 130 + d : QW + 130 + d + QT]
                prod = pool.tile([P, QT], f16, name=f"prod{k}")
                nc.vector.tensor_tensor(prod[:], sh, wts[k], mybir.AluOpType.mult)
                if acc is None:
                    acc = prod
                else:
                    nxt = pool.tile([P, QT], f16, name=f"acc{k}")
                    nc.vector.tensor_tensor(
                        nxt[:], acc[:], prod[:], mybir.AluOpType.add
                    )
                    acc = nxt

            nc.sync.dma_start(outt[:], acc[:])

    nc.compile()
    return nc


def _get_prog(name):
    if name not in _progs:
        _progs[name] = {"dense": _build_dense, "sparse": _build_sparse}[name]()
    return _progs[name]


def _run(nc, in_maps, **kwargs):
    from concourse.bass_utils import run_bass_kernel_spmd

    return run_bass_kernel_spmd(nc, in_maps, core_ids=list(range(NCORES)), **kwargs)


def _extract_diagonals(adjacency):
    """W9[t, k] = adjacency[t, t + d_k] (0 where out of range).

    Returns (W9, exact) where exact means every nonzero of adjacency lies on
    those 9 diagonals, making the stencil reproduction of the GEMM exact.
    """
    t = np.arange(S)
    W9 = np.zeros((S, NTAP), np.float32)
    for k, d in enumerate(DIAG_OFFSETS):
        s = t + d
        valid = (s >= 0) & (s < S)
        W9[valid, k] = adjacency[t[valid], s[valid]]
    exact = np.count_nonzero(adjacency) == np.count_nonzero(W9)
    return W9, exact


def _prep_dense_inmaps(sp_flat, E_flat, adjacency):
    spt = np.ascontiguousarray(sp_flat.T.reshape(KC, P, B).transpose(1, 0, 2))
    ef = np.ascontiguousarray(E_flat.reshape(KC, P).T)
    adj_bf = adjacency.astype(ml_dtypes.bfloat16)
    in_maps = []
    for m in range(NCORES):
        adjt_m = np.ascontiguousarray(adj_bf[m * TL : (m + 1) * TL, :].T)
        in_maps.append({"adjt": adjt_m, "spt": spt, "ef": ef})
    return in_maps


def _prep_sparse_inmaps(sp_flat, E_flat, W9):
    # fold the E-modulation into the tap weights: exact because the factor is
    # the power-of-two scale {1.0, -0.5}
    fac = 1.5 * E_flat - 0.5
    t = np.arange(S)
    wfold = np.empty_like(W9)  # [S, 9]
    for k, d in enumerate(DIAG_OFFSETS):
        s = np.clip(t + d, 0, S - 1)
        wfold[:, k] = W9[:, k] * fac[s]
    wfold = wfold.astype(np.float16)

    sp_pad = np.zeros((B, S + 2 * PADE + 8), np.float16)
    sp_pad[:, PADE : PADE + S] = sp_flat

    # 0/1 selector shared across cores: sel[q, p] = (p // 32 == q)
    sel = (np.arange(P)[None, :] // B == np.arange(NQ)[:, None]).astype(np.float16)

    in_maps = []
    for m in range(NCORES):
        t0 = m * TL
        spq2 = np.empty((NQ, B, 2 * QW), np.float16)
        for q in range(NQ):
            tq = t0 + q * QT
            # A copy: spq2[q,:,i] = sp_pad[:, tq+1+i] -> tap offset 129+d
            # B copy: spq2[q,:,QW+i] = sp_pad[:, tq+i] -> tap offset 130+d
            spq2[q, :, :QW] = sp_pad[:, tq + 1 : tq + 1 + QW]
            spq2[q, :, QW:] = sp_pad[:, tq : tq + QW]

        wslab = wfold[t0 : t0 + TL].reshape(NQ, QT, NTAP)
        # first NDMA taps: batch-replicated [4q*32b, NDMA, QT]
        wr = np.broadcast_to(
            wslab[:, None, :, :NDMA].transpose(0, 1, 3, 2), (NQ, B, NDMA, QT)
        )
        # remaining taps compact: row q, cols [(k-NDMA)*QT + t']
        wcsel = np.empty((NQ, WCW), np.float16)
        wcsel[:, : NBC * QT] = (
            wslab[:, :, NDMA:].transpose(0, 2, 1).reshape(NQ, NBC * QT)
        )
        wcsel[:, NBC * QT :] = sel
        in_maps.append(
            {
                "spq2": spq2.reshape(P, 2 * QW),
                "wrep": np.ascontiguousarray(wr).reshape(P, NDMA, QT),
                "wcsel": wcsel,
            }
        )
    return in_maps


def _gather_out(results):
    out = np.empty((B, S), np.float32)
    for m in range(NCORES):
        r = results[m]["out"]
        if r.shape == (P, QT):  # sparse path: unpack [32q+b, t] -> [b, q*QT+t]
            r = r.astype(np.float32).reshape(NQ, B, QT).transpose(1, 0, 2)
            r = r.reshape(B, TL)
        out[:, m * TL : (m + 1) * TL] = r
    return out


def kernel(spikes, E, adjacency):
    spikes = np.asarray(spikes, np.float32)
    E = np.asarray(E, np.float32)
    adjacency = np.asarray(adjacency, np.float32)
    sp_flat = spikes.reshape(B, S)
    E_flat = E.reshape(S)

    W9, exact = _extract_diagonals(adjacency)
    if exact:
        in_maps = _prep_sparse_inmaps(sp_flat, E_flat, W9)
        results = _run(_get_prog("sparse"), in_maps).results
    else:
        in_maps = _prep_dense_inmaps(sp_flat, E_flat, adjacency)
        results = _run(_get_prog("dense"), in_maps).results
    return _gather_out(results).reshape(B, H, W)
